# revision 20
# baseline (speedup 1.0000x reference)
"""Trainium2 Bass kernel for retrieval_knn (nn_Direct_25701084299719).

For each of N=4096 query points vs M=16384 voxels:
  - top-8 nearest voxels (L2), mean of their normals
  - cosine(mean_normal, voxel_normal) > 0.75 mask
  - score_num = sum(mask); score_sum = sum(score * mask / exp(distance))
  - out = (score_sum / max(score_num, 1)) where score_num != 0 else 0, plus valid

Sharding: data-parallel over queries across 8 NeuronCores (512 queries/core);
voxel tensors replicated. No collectives.

Device algorithm (per core; selection in fp32, streaming math in bf16):
  y[q,m]    = 2<x_q, v_m> - |v_m|^2          (K=4 augmented matmul; desc order == nearest)
  top8/idx  = nc.vector.max / max_index      (native per-partition top-8)
  xn        = sum of 8 gathered normals      (indirect-DMA gather, strided adds)
  numth[q,m]= <xn_q, n_m> - 0.75|xn_q||n_m|  (second K=4 augmented matmul)
  mask      = numth > 0  (fused count via accum_out)
  contrib   = mask * score_m * exp(-sqrt(|x_q|^2 - y))
"""

import sys

for p in ("/opt/trn_rl_repo", "/root/.axon_site/_ro/trn_rl_repo"):
    if p not in sys.path:
        sys.path.insert(0, p)

import numpy as np
from contextlib import ExitStack

import concourse.bass as bass
import concourse.mybir as mybir
from concourse import tile, masks, bacc
from concourse.bass import IndirectOffsetOnAxis
from concourse.bass_utils import run_bass_kernel_spmd

F32 = mybir.dt.float32
BF16 = mybir.dt.bfloat16
F32R = mybir.dt.float32r
U32 = mybir.dt.uint32
AF = mybir.ActivationFunctionType
OP = mybir.AluOpType
AX = mybir.AxisListType

N, M, K = 4096, 16384, 8
NCORES = 8
NQ = N // NCORES          # 512 queries per core
NT = NQ // 128            # 4 query tiles per core
CH = 512                  # m-chunk (one PSUM bank)
NCH = M // CH             # 32 chunks

_nc_cache = {}


def build_nc():
    if "nc" in _nc_cache:
        return _nc_cache["nc"]
    nc = bacc.Bacc("TRN2", target_bir_lowering=False, debug=False)
    x_d = nc.declare_dram_parameter("x", [NQ, 3], F32, isOutput=False)
    vp_d = nc.declare_dram_parameter("voxel_point", [M, 3], F32, isOutput=False)
    vn_d = nc.declare_dram_parameter("voxel_normal", [M, 3], F32, isOutput=False)
    sc_d = nc.declare_dram_parameter("score", [M], F32, isOutput=False)
    out_d = nc.declare_dram_parameter("out", [128, 2 * NT], F32, isOutput=True)
    # Internal DRAM staging. All m-indexed tensors share one fixed voxel
    # permutation m' = j*128+p <-> voxel p*128+j (from the PE-transpose build);
    # reductions over m are permutation-invariant so results are unchanged.
    Adist = nc.dram_tensor("A_dist", [4, M], F32)
    Acos = nc.dram_tensor("A_cos", [4, M], F32)  # streamed as f32r into PE
    vn4_dram = nc.dram_tensor("vn4", [M, 4], F32)     # padded normals for gather
    sc16_dram = nc.dram_tensor("sc16", [M], BF16)     # permuted bf16 scores

    with tile.TileContext(nc) as tc, ExitStack() as ctx:
        cpool = ctx.enter_context(tc.tile_pool(name="const", bufs=1))
        big = ctx.enter_context(tc.tile_pool(name="big", bufs=1))
        prep = ctx.enter_context(tc.tile_pool(name="prep", bufs=2))
        chk = ctx.enter_context(tc.tile_pool(name="chk", bufs=3))
        pp = ctx.enter_context(tc.tile_pool(name="pp", bufs=2, space="PSUM"))
        sm = ctx.enter_context(tc.tile_pool(name="sm", bufs=2))

        ident = cpool.tile([128, 128], F32)
        masks.make_identity(nc, ident[:])

        # ---------------- prep: voxel-side tensors ----------------
        vn_sb = prep.tile([128, 384], F32)
        nc.sync.dma_start(vn_sb[:], vn_d[:].rearrange("(p j) d -> p (j d)", p=128))
        vp_sb = prep.tile([128, 384], F32)
        nc.sync.dma_start(vp_sb[:], vp_d[:].rearrange("(p j) d -> p (j d)", p=128))

        def build_A(src_sb, A_dram, row3_from_sq, keep4_sb=None):
            """A rows 0-2: coordinate d in m' order; row 3 from sum of squares.
            keep4_sb: optionally collect rows 0-2 into a [128, 128*4] row-padded
            layout (gather table: partition j holds voxels j*128..j*128+127)."""
            v3 = src_sb[:].rearrange("p (j d) -> p j d", d=3)
            for d in range(3):
                ps = pp.tile([128, 128], F32, tag="pt")
                nc.tensor.transpose(ps[:], v3[:, :, d], ident[:])
                tmp = prep.tile([128, 128], F32, tag="atmp")
                nc.scalar.activation(tmp[:], ps[:], AF.Copy)
                nc.sync.dma_start(
                    A_dram[d : d + 1, :].rearrange("o (j p) -> (o j) p", j=128), tmp[:]
                )
                if keep4_sb is not None:
                    k4 = keep4_sb[:].rearrange("p (c e) -> p c e", e=4)
                    nc.vector.tensor_copy(k4[:, :, d], tmp[:])
            sq = prep.tile([128, 384], F32, tag="asq")
            nc.scalar.activation(sq[:], src_sb[:], AF.Square)
            s3 = sq[:].rearrange("p (j d) -> p j d", d=3)
            ss = prep.tile([128, 128], F32, tag="ass")
            nc.vector.tensor_tensor(ss[:], s3[:, :, 0], s3[:, :, 1], OP.add)
            nc.vector.tensor_tensor(ss[:], ss[:], s3[:, :, 2], OP.add)
            r3 = prep.tile([128, 128], F32, tag="ar3")
            row3_from_sq(r3, ss)
            ps = pp.tile([128, 128], F32, tag="pt")
            nc.tensor.transpose(ps[:], r3[:], ident[:])
            tmp = prep.tile([128, 128], F32, tag="atmp")
            nc.scalar.activation(tmp[:], ps[:], AF.Copy)
            nc.sync.dma_start(
                A_dram[3:4, :].rearrange("o (j p) -> (o j) p", j=128), tmp[:]
            )

        # A_dist row3 = -|v|^2 ;  A_cos row3 = +0.75*|n| = sqrt(0.5625*|n|^2)
        build_A(vp_sb, Adist,
                lambda r3, ss: nc.vector.tensor_scalar_mul(r3[:], ss[:], -1.0))
        vn4_sb = prep.tile([128, 512], F32)
        nc.vector.memset(vn4_sb[:], 0.0)
        build_A(vn_sb, Acos,
                lambda r3, ss: nc.scalar.activation(r3[:], ss[:], AF.Sqrt, scale=0.5625),
                keep4_sb=vn4_sb)
        # vn4[j*128+p, 0:3] = vn in m' order, rows padded to 16B for the gather
        nc.sync.dma_start(
            vn4_dram[:].rearrange("(j p) e -> j (p e)", j=128), vn4_sb[:]
        )

        # bf16 permuted ln(score) row in DRAM, then partition-broadcast.
        # Folding score into the exponent (exp(ln s - d)) removes one full
        # elementwise multiply from the stream.
        sc_pm = prep.tile([128, 128], F32)
        nc.sync.dma_start(sc_pm[:], sc_d[:].rearrange("(p j) -> p j", p=128))
        ln_pm = prep.tile([128, 128], F32)
        nc.scalar.activation(ln_pm[:], sc_pm[:], AF.Ln)
        psT = pp.tile([128, 128], F32, tag="pt")
        nc.tensor.transpose(psT[:], ln_pm[:], ident[:])
        scT16 = prep.tile([128, 128], BF16)
        nc.scalar.activation(scT16[:], psT[:], AF.Copy)
        nc.sync.dma_start(sc16_dram[:].rearrange("(j p) -> j p", j=128), scT16[:])
        lns_bc = big.tile([128, M], BF16, tag="sbc")
        nc.sync.dma_start(
            lns_bc[:], sc16_dram[:].rearrange("(o m) -> o m", o=1).partition_broadcast(128)
        )

        # ---------------- prep: query-side tensors ----------------
        xxs, lts = [], []
        for t in range(NT):
            xt = cpool.tile([128, 3], F32, tag=f"xt{t}")
            nc.sync.dma_start(xt[:], x_d[t * 128 : (t + 1) * 128, :])
            sqx = sm.tile([128, 3], F32, tag="sqx")
            nc.scalar.activation(sqx[:], xt[:], AF.Square)
            xx = cpool.tile([128, 1], F32, tag=f"xx{t}")
            nc.vector.tensor_reduce(xx[:], sqx[:], AX.X, OP.add)
            # lhsT rows [2x0;2x1;2x2;1]: 0.5 in col 3 pre-transpose, Copy(scale=2)
            xt4 = sm.tile([128, 4], F32, tag="xt4")
            nc.vector.tensor_copy(xt4[:, 0:3], xt[:])
            nc.vector.memset(xt4[:, 3:4], 0.5)
            ps = pp.tile([128, 128], F32, tag="pt")
            nc.tensor.transpose(ps[0:4, 0:128], xt4[:], ident[:])
            lt = cpool.tile([4, 128], F32, tag=f"lt{t}")
            nc.scalar.activation(lt[:], ps[0:4, 0:128], AF.Copy, scale=2.0)
            xxs.append(xx)
            lts.append(lt)

        out_sb = cpool.tile([128, 2 * NT], F32)

        # ---------------- main loop over query tiles ----------------
        for t in range(NT):
            # Phase 1: y = 2<x,v> - |v|^2 (fp32), full row resident
            y = big.tile([128, M], F32, tag="y")
            for c in range(NCH):
                ra = chk.tile([4, CH], F32, tag="ra")
                nc.sync.dma_start(ra[:], Adist[:, c * CH : (c + 1) * CH])
                ps = pp.tile([128, CH], F32, tag="pm")
                nc.tensor.matmul(ps[:], lhsT=lts[t][:], rhs=ra[:], start=True, stop=True)
                nc.scalar.activation(y[:, c * CH : (c + 1) * CH], ps[:], AF.Copy)

            # Selection: native top-8 + indices, gather padded normals
            top8 = sm.tile([128, 8], F32, tag="top8")
            nc.vector.max(top8[:], y[:])
            idx8 = sm.tile([128, 8], U32, tag="idx8")
            nc.vector.max_index(idx8[:], top8[:], y[:])
            # HW DGE consumes one offset per partition per instruction, so
            # issue one gather per neighbor rank.
            g = sm.tile([128, 32], F32, tag="gat")
            g3 = g[:].rearrange("p (i e) -> p i e", e=4)
            for i in range(8):
                nc.gpsimd.indirect_dma_start(
                    g3[:, i, :], None, vn4_dram[:],
                    IndirectOffsetOnAxis(ap=idx8[:, i : i + 1], axis=0),
                )
            # xn = sum of the 8 gathered normal rows (tree of strided adds)
            h16 = sm.tile([128, 16], F32, tag="h16")
            nc.vector.tensor_tensor(h16[:], g[:, 0:16], g[:, 16:32], OP.add)
            h8 = sm.tile([128, 8], F32, tag="h8")
            nc.vector.tensor_tensor(h8[:], h16[:, 0:8], h16[:, 8:16], OP.add)
            xn4 = sm.tile([128, 4], F32, tag="xn4")
            nc.vector.tensor_tensor(xn4[:], h8[:, 0:4], h8[:, 4:8], OP.add)

            # lhsT for cos matmul: [xn0;xn1;xn2;-|xn|] (xn = 8*mean, scale-free)
            sqn = sm.tile([128, 3], F32, tag="sqn")
            nc.vector.tensor_tensor(sqn[:], xn4[:, 0:3], xn4[:, 0:3], OP.mult)
            nrm2 = sm.tile([128, 1], F32, tag="nrm2")
            nc.vector.tensor_reduce(nrm2[:], sqn[:], AX.X, OP.add)
            nc.scalar.activation(xn4[:, 3:4], nrm2[:], AF.Sqrt)
            nc.vector.tensor_scalar_mul(xn4[:, 3:4], xn4[:, 3:4], -1.0)
            psl = pp.tile([128, 128], F32, tag="pt")
            nc.tensor.transpose(psl[0:4, 0:128], xn4[:], ident[:])
            ltc = sm.tile([4, 128], F32R, tag="ltc")
            nc.vector.tensor_copy(ltc[:], psl[0:4, 0:128])

            # Phase 3 (batched per ACT function to avoid LUT reloads):
            #   d = sqrt(xx - y)            [ACT Sqrt x32, bf16]
            #   z = ln(s) - d               [DVE, bf16 2x, in place]
            #   es = exp(z), accum -> accE  [ACT Exp x32, in place]
            #   sg = sign(numth), accum     [ACT Sign x32, accE-style]
            #   prod = sg * es, reduce      [DVE bf16]
            # count = (sum(sg) + M)/2 ; ssum = (sum(sg*es) + sum(es))/2
            dfull = big.tile([128, M], BF16, tag="dfull")
            for c in range(NCH):
                cs = slice(c * CH, (c + 1) * CH)
                nc.scalar.activation(
                    dfull[:, cs], y[:, cs], AF.Sqrt, bias=xxs[t][:], scale=-1.0
                )
            for c in range(NCH):
                cs = slice(c * CH, (c + 1) * CH)
                nc.vector.tensor_tensor(
                    dfull[:, cs], lns_bc[:, cs], dfull[:, cs], OP.subtract
                )
            accE = sm.tile([128, NCH], F32, tag="accE")
            for c in range(NCH):
                cs = slice(c * CH, (c + 1) * CH)
                nc.scalar.activation(
                    dfull[:, cs], dfull[:, cs], AF.Exp,
                    accum_out=accE[:, c : c + 1],
                )
            accSg = sm.tile([128, NCH], F32, tag="accSg")
            accS = sm.tile([128, NCH], F32, tag="accS")
            for c in range(NCH):
                cs = slice(c * CH, (c + 1) * CH)
                rc = chk.tile([4, CH], F32R, tag="rc")
                nc.sync.dma_start(rc[:], Acos[:, cs].bitcast(F32R))
                psn = pp.tile([128, CH], F32, tag="pm")
                nc.tensor.matmul(psn[:], lhsT=ltc[:], rhs=rc[:], start=True, stop=True)
                sg = chk.tile([128, CH], BF16, tag="sg")
                nc.scalar.activation(
                    sg[:], psn[:], AF.Sign, accum_out=accSg[:, c : c + 1]
                )
                prod = chk.tile([128, CH], BF16, tag="prod")
                nc.vector.tensor_tensor(prod[:], sg[:], dfull[:, cs], OP.mult)
                nc.vector.tensor_reduce(accS[:, c : c + 1], prod[:], AX.X, OP.add)

            cnt = sm.tile([128, 1], F32, tag="cnt")
            nc.vector.tensor_reduce(cnt[:], accSg[:], AX.X, OP.add)
            nc.vector.tensor_scalar(cnt[:], cnt[:], float(M), 0.5, OP.add, OP.mult)
            ssum = sm.tile([128, 1], F32, tag="ssum")
            nc.vector.tensor_reduce(ssum[:], accS[:], AX.X, OP.add)
            sse = sm.tile([128, 1], F32, tag="sse")
            nc.vector.tensor_reduce(sse[:], accE[:], AX.X, OP.add)
            nc.vector.tensor_tensor(ssum[:], ssum[:], sse[:], OP.add)
            nc.vector.tensor_scalar_mul(ssum[:], ssum[:], 0.5)
            den = sm.tile([128, 1], F32, tag="den")
            nc.vector.tensor_scalar_max(den[:], cnt[:], 1.0)
            rden = sm.tile([128, 1], F32, tag="rden")
            nc.vector.reciprocal(rden[:], den[:])
            vld = sm.tile([128, 1], F32, tag="vld")
            nc.vector.tensor_scalar(vld[:], cnt[:], 0.5, None, OP.is_gt)
            fld = sm.tile([128, 1], F32, tag="fld")
            nc.vector.tensor_tensor(fld[:], ssum[:], rden[:], OP.mult)
            nc.vector.tensor_tensor(fld[:], fld[:], vld[:], OP.mult)
            nc.vector.tensor_copy(out_sb[:, t : t + 1], fld[:])
            nc.vector.tensor_copy(out_sb[:, NT + t : NT + t + 1], cnt[:])

        nc.sync.dma_start(out_d[:, :], out_sb[:])

    nc.compile()
    _nc_cache["nc"] = nc
    return nc


def make_in_maps(x_world, voxel_point, voxel_normal, score):
    x = np.ascontiguousarray(np.asarray(x_world, np.float32).reshape(N, 3))
    vp = np.ascontiguousarray(np.asarray(voxel_point, np.float32).reshape(M, 3))
    vn = np.ascontiguousarray(np.asarray(voxel_normal, np.float32).reshape(M, 3))
    sc = np.ascontiguousarray(np.asarray(score, np.float32).reshape(M))
    return [
        {
            "x": np.ascontiguousarray(x[c * NQ : (c + 1) * NQ]),
            "voxel_point": vp,
            "voxel_normal": vn,
            "score": sc,
        }
        for c in range(NCORES)
    ]


def decode_outputs(results):
    fields, cnts = [], []
    for r in results:
        o = np.asarray(r["out"])  # [128, 8]
        fields.append(o[:, 0:NT].T.reshape(NQ))
        cnts.append(o[:, NT : 2 * NT].T.reshape(NQ))
    field = np.concatenate(fields).astype(np.float32)
    cnt = np.concatenate(cnts)
    valid = cnt > 0.5
    return field, valid


def kernel(x_world, voxel_point, voxel_normal, score):
    nc = build_nc()
    in_maps = make_in_maps(x_world, voxel_point, voxel_normal, score)
    res = run_bass_kernel_spmd(nc, in_maps, core_ids=list(range(NCORES)))
    return decode_outputs(res.results)

